# revision 7
# baseline (speedup 1.0000x reference)
"""MoE (top-2 routing, 8 experts) Trainium2 kernel — fp8 DoubleRow version.

Strategy (expert-parallel, matches the sharding hint):
  - Gating (x @ Wg + bg, top-2, softmax) is computed on the host in float64.
    The top-2/3rd logit gap for these inputs is >=1.6e-5, far above fp32
    rounding noise, so the host selection matches the fp32 reference exactly.
  - Tokens are dispatched by expert id: core e receives the tokens routed to
    expert e (padded to a uniform capacity C), plus expert e's weights.
  - Each core runs a Bass/Tile kernel computing
        yT = (relu(x @ W1 + b1) @ W2 + b2)^T      (shape [O, C])
  - The host combines: out[t] = sum_k gate[t,k] * y_{expert_k(t)}[t].

Compute scheme: fp8e4m3 hi/lo split with DoubleRow matmuls.
  Every operand A (x, W1, h, W2) is represented as A_hi + A_lo, both e4m3
  (A_lo = e4m3(A - A_hi)), with weights pre-scaled by 2^6 and h stored at
  2^HS so everything sits in e4m3's normal range. Each matmul product is
  computed in three passes accumulated in PSUM:
        A@B ~= A_hi@B_hi + A_hi@B_lo + A_lo@B_hi
  (the dropped lo@lo term is ~1e-4 relative). DoubleRow contracts 2 k-tiles
  (256) per instruction at 0.5 cycles/row, so the 3-pass scheme costs 0.75x
  a single bf16 pass while keeping ~bf16 accuracy (measured 2e-3 end to end).
  All scales are powers of two folded into the ACT-engine epilogues (relu is
  positively homogeneous), so no extra scaling ops are needed.
"""

import numpy as np

T, D, H, O, E, TOPK = 4096, 1024, 2048, 1024, 8, 2
P = 128
DK, HK, HT, OT = D // P, H // P, H // P, O // P

SW = 6   # W1/W2 stored as e4m3(W * 2^SW)
HS = 5   # h stored as 2^HS * relu(x@W1 + b1)  (max |h|*2^5 ~ 96 << 240)

NCH = 3  # x/h processed in NCH equal token chunks

_BUILD_CACHE = {}


def _capacity(max_load):
    """Uniform per-core capacity: multiple of NCH*16 so chunks are equal and
    16-aligned."""
    g = NCH * 16
    return max(768, -(-max_load // g) * g)


def _build(C):
    import concourse.mybir as mybir
    import concourse.tile as tile
    from concourse import bacc

    f32 = mybir.dt.float32
    f8 = mybir.dt.float8e4
    f32r = mybir.dt.float32r
    DR = mybir.MatmulPerfMode.DoubleRow

    assert C % (NCH * 16) == 0
    cn = C // NCH
    chunks = [(i * cn, cn) for i in range(NCH)]

    nc = bacc.Bacc("TRN2", target_bir_lowering=False)
    # chunk-major fp8 x (hi/lo packed) so each chunk is one full-rate DMA
    x8 = nc.dram_tensor("x8", (NCH, P, 2, DK, cn), f8, kind="ExternalInput")
    w1 = nc.dram_tensor("w1", (P, HT, 2, DK, P), f8, kind="ExternalInput")
    w2 = nc.dram_tensor("w2", (P, OT, 2, HK, P), f8, kind="ExternalInput")
    b1s = nc.dram_tensor("b1s", (P, HT), f32, kind="ExternalInput")  # 2^HS*b1
    b2s = nc.dram_tensor("b2s", (P, OT), f32, kind="ExternalInput")
    yT = nc.dram_tensor("yT", (O, C), f32, kind="ExternalOutput")

    with tile.TileContext(nc) as tc:
        with (
            tc.tile_pool(name="const", bufs=1) as constp,
            tc.tile_pool(name="main", bufs=1) as mainp,
            tc.tile_pool(name="tmp", bufs=4) as tmpp,
            tc.tile_pool(name="yp", bufs=3) as yp,
            tc.tile_pool(name="ps", bufs=7, space="PSUM") as psp,
            tc.tile_pool(name="warmp", bufs=1, space="PSUM") as warmp,
        ):
            # PE warm-up: dummy f32r matmuls keep the PE busy through the
            # initial DMA window so the HAM clock is fully ramped (3us of
            # continuous execution) when real work arrives.
            warm_w = constp.tile([P, P], f32r, name="warm_w")
            warm_x = constp.tile([P, 256], f32r, name="warm_x")
            nc.vector.memset(warm_w[:].bitcast(mybir.dt.uint32), 0)
            nc.vector.memset(warm_x[:].bitcast(mybir.dt.uint32), 0)
            warm_ps = warmp.tile([P, 256], f32, name="warm_ps")
            for _ in range(16):
                nc.tensor.matmul(
                    warm_ps[:, :], warm_w[:, :], warm_x[:, :],
                    start=True, stop=True,
                )

            # Weights: fully resident in SBUF (hi+lo = 64KB/partition).
            # Few large DMAs; the first covers the startup-critical tiles.
            w1_sb = mainp.tile([P, HT, 2, DK, P], f8)
            w2_sb = mainp.tile([P, OT, 2, HK, P], f8)
            for a, b in ((0, 2), (2, 8), (8, 16)):
                nc.sync.dma_start(w1_sb[:, a:b], w1[:, a:b])
            for a, b in ((0, 4), (4, 8)):
                nc.sync.dma_start(w2_sb[:, a:b], w2[:, a:b])

            # x: one hi+lo DMA per chunk on the gpsimd queue.
            x_sb = mainp.tile([P, NCH, 2, DK, cn], f8)
            for i in range(NCH):
                nc.gpsimd.dma_start(x_sb[:, i], x8[i])

            b1_sb = constp.tile([P, HT], f32)
            nc.scalar.dma_start(b1_sb[:], b1s[:])
            b2_sb = constp.tile([P, OT], f32)
            nc.scalar.dma_start(b2_sb[:], b2s[:])

            hh_sb = mainp.tile([P, HT, C], f8)
            hl_sb = mainp.tile([P, HT, C], f8)

            # Phase 1: h[ht] = relu(2^-1 * ps + 2^HS*b1),  ps = 2^6 x@W1
            # chunk-outer: chunk 0 feeds the first 16 groups, so only the
            # first x DMA gates startup.
            for ci, (c0, cnn) in enumerate(chunks):
                for ht in range(HT):
                    ps = psp.tile(
                        [P, 512], f32, tag="ps", name=f"ps1_{ht}_{ci}"
                    )[:, :cnn]
                    n = 0
                    for wi, xi in ((0, 0), (1, 0), (0, 1)):
                        for j in range(DK // 2):
                            nc.tensor.matmul(
                                ps,
                                w1_sb[:, ht, wi, 2 * j : 2 * j + 2, :],
                                x_sb[:, ci, xi, 2 * j : 2 * j + 2, :],
                                start=(n == 0),
                                stop=(n == 3 * DK // 2 - 1),
                                perf_mode=DR,
                            )
                            n += 1
                    tmp = tmpp.tile([P, 512], f32, tag="tmp", name=f"t_{ht}_{ci}")[
                        :, :cnn
                    ]
                    nc.scalar.activation(
                        tmp,
                        ps,
                        mybir.ActivationFunctionType.Relu,
                        bias=b1_sb[:, ht : ht + 1],
                        scale=float(2.0 ** (HS - SW)),
                    )
                    nc.vector.tensor_copy(hh_sb[:, ht, c0 : c0 + cnn], tmp)
                    nc.vector.tensor_tensor(
                        hl_sb[:, ht, c0 : c0 + cnn],
                        tmp,
                        hh_sb[:, ht, c0 : c0 + cnn],
                        mybir.AluOpType.subtract,
                    )

            # Phase 2: y[ot] = 2^-(HS+SW) * ps2 + b2,  ps2 = 2^(HS+SW) h@W2
            for ot in range(OT):
                y_sb = yp.tile([P, C], f32, tag="y", name=f"y_{ot}")
                for ci, (c0, cnn) in enumerate(reversed(chunks)):
                    ps = psp.tile(
                        [P, 512], f32, tag="ps", name=f"ps2_{ot}_{ci}"
                    )[:, :cnn]
                    n = 0
                    for wi, hsb in ((0, hh_sb), (1, hh_sb), (0, hl_sb)):
                        for j in range(HK // 2):
                            nc.tensor.matmul(
                                ps,
                                w2_sb[:, ot, wi, 2 * j : 2 * j + 2, :],
                                hsb[:, 2 * j : 2 * j + 2, c0 : c0 + cnn],
                                start=(n == 0),
                                stop=(n == 3 * HK // 2 - 1),
                                perf_mode=DR,
                            )
                            n += 1
                    nc.scalar.activation(
                        y_sb[:, c0 : c0 + cnn],
                        ps,
                        mybir.ActivationFunctionType.Identity,
                        bias=b2_sb[:, ot : ot + 1],
                        scale=float(2.0 ** (-HS - SW)),
                    )
                    if ot == OT - 1:
                        # tail: stagger the final output in per-chunk pieces
                        nc.scalar.dma_start(
                            yT[ot * P : (ot + 1) * P, c0 : c0 + cnn],
                            y_sb[:, c0 : c0 + cnn],
                        )
                if ot < OT - 1:
                    nc.scalar.dma_start(yT[ot * P : (ot + 1) * P, :], y_sb[:])

    nc.compile()
    return nc


LAST_BUILD_KEY = None


def _get_built(C):
    global LAST_BUILD_KEY
    key = (C,)
    if key not in _BUILD_CACHE:
        _BUILD_CACHE[key] = _build(C)
    LAST_BUILD_KEY = key
    return _BUILD_CACHE[key]


_RUNNER_CACHE = {}
_WEIGHT_CACHE = {}


def _get_runner(C):
    """Reusable jitted SPMD executable for the bass program (compile once)."""
    key = (C,)
    if key in _RUNNER_CACHE:
        return _RUNNER_CACHE[key]

    import jax
    import concourse.mybir as mybir
    from concourse import bass2jax
    from jax.experimental.shard_map import shard_map
    from jax.sharding import Mesh, NamedSharding, PartitionSpec

    nc = _get_built(C)
    bass2jax.install_neuronx_cc_hook()

    partition_name = (
        nc.partition_id_tensor.name if nc.partition_id_tensor else None
    )
    in_names, out_names, out_avals = [], [], []
    for alloc in nc.m.functions[0].allocations:
        if not isinstance(alloc, mybir.MemoryLocationSet):
            continue
        name = alloc.memorylocations[0].name
        if alloc.kind == "ExternalInput":
            if name != partition_name:
                in_names.append(name)
        elif alloc.kind == "ExternalOutput":
            out_names.append(name)
            out_avals.append(
                jax.core.ShapedArray(
                    tuple(alloc.tensor_shape), mybir.dt.np(alloc.dtype)
                )
            )
    all_names = list(in_names) + list(out_names) + (
        [partition_name] if partition_name else []
    )

    def _body(*args):
        operands = list(args)
        if partition_name is not None:
            operands.append(bass2jax.partition_id_tensor())
        outs = bass2jax._bass_exec_p.bind(
            *operands,
            out_avals=tuple(out_avals),
            in_names=tuple(all_names),
            out_names=tuple(out_names),
            lowering_input_output_aliases=(),
            sim_require_finite=True,
            sim_require_nnan=True,
            nc=nc,
        )
        return tuple(outs)

    devices = jax.devices()[:E]
    mesh = Mesh(np.asarray(devices), ("core",))
    n_io = len(in_names) + len(out_names)
    fn = jax.jit(
        shard_map(
            _body,
            mesh=mesh,
            in_specs=(PartitionSpec("core"),) * n_io,
            out_specs=(PartitionSpec("core"),) * len(out_names),
            check_rep=False,
        ),
        keep_unused=True,
    )
    sharding = NamedSharding(mesh, PartitionSpec("core"))
    # Zero-filled output parameter buffers, device-resident. Not donated: the
    # kernel writes every element of its outputs, so reuse across calls is
    # safe.
    zeros = [
        jax.device_put(
            np.zeros((E * av.shape[0], *av.shape[1:]), av.dtype), sharding
        )
        for av in out_avals
    ]
    runner = {
        "fn": fn,
        "in_names": in_names,
        "out_names": out_names,
        "sharding": sharding,
        "zeros": zeros,
    }
    _RUNNER_CACHE[key] = runner
    return runner


def _weights_fingerprint(arrays):
    import hashlib

    h = hashlib.sha1()
    for k in sorted(arrays):
        a = np.ascontiguousarray(arrays[k])
        h.update(k.encode())
        h.update(str(a.shape).encode())
        flat = a.view(np.uint8).reshape(-1)
        h.update(flat[:: max(1, flat.size // 262144)].tobytes())  # ~256KB sample
        h.update(flat[-4096:].tobytes())
    return h.hexdigest()


def _device_weights(runner, key, arrays):
    """device_put the per-core-stacked weight arrays once, keyed by content."""
    import jax

    fp = (key, _weights_fingerprint(arrays))
    if fp not in _WEIGHT_CACHE:
        _WEIGHT_CACHE.clear()  # keep at most one weight set resident
        _WEIGHT_CACHE[fp] = {
            k: jax.device_put(v, runner["sharding"]) for k, v in arrays.items()
        }
    return _WEIGHT_CACHE[fp]


def _route(x, Wg, bg):
    """Host gating in float64; returns per-expert token ids and gate weights."""
    logits = x.astype(np.float64) @ Wg.astype(np.float64) + bg.astype(np.float64)
    order = np.argsort(-logits, axis=1, kind="stable")
    top2 = order[:, :TOPK]  # [T, 2]
    v = np.take_along_axis(logits, top2, axis=1)
    ex = np.exp(v - v.max(axis=1, keepdims=True))
    g = (ex / ex.sum(axis=1, keepdims=True)).astype(np.float32)  # [T, 2]
    ids, gates = [], []
    for e in range(E):
        sel = top2 == e  # [T, 2]
        te = np.where(sel.any(axis=1))[0]
        ge = np.where(sel[te, 0], g[te, 0], g[te, 1])
        ids.append(te)
        gates.append(ge.astype(np.float32))
    return ids, gates


def _f8():
    import ml_dtypes

    return np.dtype(ml_dtypes.float8_e4m3)


def _split_f8(a):
    """Return (hi, lo) e4m3 arrays with hi + lo ~= a."""
    f8 = _f8()
    hi = a.astype(f8)
    lo = (a - hi.astype(np.float32)).astype(f8)
    return hi, lo


def _prep_weights(W1, b1, W2, b2):
    """Quantize + lay out weights for the kernel, stacked per core.

    w1 tile layout: [p, ht, dk, m] = W1s[dk*128+p, ht*128+m]
    w2 tile layout: [p, ot, hk, m] = W2s[hk*128+p, ot*128+m]
    """
    s = np.float32(2.0**SW)
    arrs = {}
    # [E, D, H] -> [E, dk, p, ht, m] -> [E*p, ht, dk, m]
    W1s = (W1 * s).reshape(E, DK, P, HT, P)
    W2s = (W2 * s).reshape(E, HK, P, OT, P)
    for name, Ws in (("w1", W1s), ("w2", W2s)):
        hi, lo = _split_f8(Ws.astype(np.float32))
        # [e, k_tiles, p, out_tiles, m] -> [e, p, out_tiles, {hi,lo}, k_tiles, m]
        t = np.stack(
            (hi.transpose(0, 2, 3, 1, 4), lo.transpose(0, 2, 3, 1, 4)), axis=3
        )
        arrs[name] = np.ascontiguousarray(
            t.reshape(E * P, t.shape[2], 2, t.shape[4], P)
        )
    arrs["b1s"] = np.ascontiguousarray(
        (b1 * np.float32(2.0**HS)).reshape(E, HT, P).transpose(0, 2, 1).reshape(E * P, HT)
    ).astype(np.float32)
    arrs["b2s"] = np.ascontiguousarray(
        b2.reshape(E, OT, P).transpose(0, 2, 1).reshape(E * P, OT)
    ).astype(np.float32)
    return arrs


def _is_axon():
    try:
        from concourse._compat import axon_active

        return bool(axon_active())
    except Exception:  # noqa: BLE001
        return False


def _build_x_global(C, ids, x):
    """Chunk-major fp8 hi/lo-packed x dispatch array, stacked per core.

    Returns x8_g of shape [E*NCH, P, 2, DK, cn]; core e's slice is
    [e*NCH:(e+1)*NCH] with layout [chunk, p, hi/lo, dk, c].
    """
    cn = C // NCH
    f8 = _f8()
    x8_g = np.zeros((E, NCH, P, 2, DK, cn), f8)
    for e in range(E):
        te = ids[e]
        if len(te) == 0:
            continue
        xt = np.zeros((C, DK, P), np.float32)
        xt[: len(te)] = x[te].reshape(len(te), DK, P)
        hi, lo = _split_f8(xt)
        # [C, dk, p] -> [nch, cn, dk, p] -> [nch, p, dk, cn]
        x8_g[e, :, :, 0] = hi.reshape(NCH, cn, DK, P).transpose(0, 3, 2, 1)
        x8_g[e, :, :, 1] = lo.reshape(NCH, cn, DK, P).transpose(0, 3, 2, 1)
    return np.ascontiguousarray(x8_g.reshape(E * NCH, P, 2, DK, cn))


def _run_axon(C, ids, x, warrs):
    """Fast path: cached jitted SPMD executable, device-resident weights."""
    import jax

    runner = _get_runner(C)
    dev_w = _device_weights(runner, (C,), warrs)

    x8_g = _build_x_global(C, ids, x)
    x8_dev = jax.device_put(x8_g, runner["sharding"])

    operands = []
    for name in runner["in_names"]:
        if name == "x8":
            operands.append(x8_dev)
        else:
            operands.append(dev_w[name])
    operands.extend(runner["zeros"])
    outs = runner["fn"](*operands)
    return np.asarray(outs[runner["out_names"].index("yT")])  # [E*O, C]


def _run_native(C, ids, x, warrs):
    """Fallback for non-axon environments: bass_utils native NRT runner."""
    from concourse.bass_utils import run_bass_kernel_spmd

    nc = _get_built(C)
    x8_g = _build_x_global(C, ids, x)
    in_maps = []
    for e in range(E):
        m = {
            "x8": np.ascontiguousarray(x8_g[e * NCH : (e + 1) * NCH]),
        }
        for k, v in warrs.items():
            m[k] = np.ascontiguousarray(v[e * P : (e + 1) * P])
        in_maps.append(m)
    res = run_bass_kernel_spmd(nc, in_maps, core_ids=list(range(E)))
    return np.concatenate([res.results[e]["yT"] for e in range(E)], axis=0)


# Above this capacity the working set (x + h + y tiles at current pool
# depths) overflows SBUF; heavier routing skew runs as multiple batches.
_MAX_C = 1920

FALLBACK_USED = False  # set when the numpy emergency path ran (device down)


def _run_device(C, bids, x, warrs, W1, b1, W2, b2):
    """Run the bass kernel on the 8 cores, with one retry after a device
    error and a loud numpy fallback if the accelerator is unrecoverable."""
    for attempt in range(2):
        try:
            if _is_axon():
                return _run_axon(C, bids, x, warrs)
            return _run_native(C, bids, x, warrs)
        except Exception as ex:  # noqa: BLE001
            print(
                f"kernel: device run failed (attempt {attempt}): "
                f"{type(ex).__name__}: {str(ex)[:200]}",
                flush=True,
            )
            # Device arrays / executables may be poisoned; rebuild them.
            _RUNNER_CACHE.clear()
            _WEIGHT_CACHE.clear()
            try:
                import jax

                jax.clear_caches()
            except Exception:  # noqa: BLE001
                pass
    global FALLBACK_USED
    FALLBACK_USED = True
    print(
        "kernel: WARNING - accelerator unavailable after retries; "
        "computing this batch on the host (numpy) so the result is correct",
        flush=True,
    )
    yT_g = np.zeros((E * O, C), np.float32)
    for e in range(E):
        te = bids[e]
        if len(te) == 0:
            continue
        h = np.maximum(x[te] @ W1[e] + b1[e], 0.0)
        yT_g[e * O : (e + 1) * O, : len(te)] = (h @ W2[e] + b2[e]).T
    return yT_g


def kernel(x, Wg, bg, W1, b1, W2, b2):
    x = np.ascontiguousarray(np.asarray(x, np.float32))
    Wg = np.asarray(Wg, np.float32)
    bg = np.asarray(bg, np.float32)
    W1 = np.ascontiguousarray(np.asarray(W1, np.float32))
    b1 = np.ascontiguousarray(np.asarray(b1, np.float32))
    W2 = np.ascontiguousarray(np.asarray(W2, np.float32))
    b2 = np.ascontiguousarray(np.asarray(b2, np.float32))

    assert x.shape[1] == D and Wg.shape == (D, E)
    assert W1.shape == (E, D, H) and W2.shape == (E, H, O)

    ids, gates = _route(x, Wg, bg)

    warrs = _prep_weights(W1, b1, W2, b2)

    out = np.zeros((x.shape[0], O), np.float32)
    max_load = max(len(te) for te in ids)
    n_batches = -(-max_load // _MAX_C)
    for b in range(n_batches):
        bids = [te[b * _MAX_C : (b + 1) * _MAX_C] for te in ids]
        C = _capacity(max(len(te) for te in bids))
        yT_g = _run_device(C, bids, x, warrs, W1, b1, W2, b2)
        for e in range(E):
            te = bids[e]
            ge = gates[e][b * _MAX_C : (b + 1) * _MAX_C]
            ye = yT_g[e * O : e * O + O, : len(te)].T  # [n_e, O]
            out[te] += ge[:, None] * ye
    return out


# revision 8
# speedup vs baseline: 1.0599x; 1.0599x over previous
"""MoE (top-2 routing, 8 experts) Trainium2 kernel — fp8 DoubleRow version.

Strategy (expert-parallel, matches the sharding hint):
  - Gating (x @ Wg + bg, top-2, softmax) is computed on the host in float64.
    The top-2/3rd logit gap for these inputs is >=1.6e-5, far above fp32
    rounding noise, so the host selection matches the fp32 reference exactly.
  - Tokens are dispatched by expert id: core e receives the tokens routed to
    expert e (padded to a uniform capacity C), plus expert e's weights.
  - Each core runs a Bass/Tile kernel computing
        yT = (relu(x @ W1 + b1) @ W2 + b2)^T      (shape [O, C])
  - The host combines: out[t] = sum_k gate[t,k] * y_{expert_k(t)}[t].

Compute scheme: fp8e4m3 hi/lo split with DoubleRow matmuls.
  Every operand A (x, W1, h, W2) is represented as A_hi + A_lo, both e4m3
  (A_lo = e4m3(A - A_hi)), with weights pre-scaled by 2^6 and h stored at
  2^HS so everything sits in e4m3's normal range. Each matmul product is
  computed in three passes accumulated in PSUM:
        A@B ~= A_hi@B_hi + A_hi@B_lo + A_lo@B_hi
  (the dropped lo@lo term is ~1e-4 relative). DoubleRow contracts 2 k-tiles
  (256) per instruction at 0.5 cycles/row, so the 3-pass scheme costs 0.75x
  a single bf16 pass while keeping ~bf16 accuracy (measured 2e-3 end to end).
  All scales are powers of two folded into the ACT-engine epilogues (relu is
  positively homogeneous), so no extra scaling ops are needed.
"""

import numpy as np

T, D, H, O, E, TOPK = 4096, 1024, 2048, 1024, 8, 2
P = 128
DK, HK, HT, OT = D // P, H // P, H // P, O // P

SW = 6   # W1/W2 stored as e4m3(W * 2^SW)
HS = 5   # h stored as 2^HS * relu(x@W1 + b1)  (max |h|*2^5 ~ 96 << 240)

NCH = 3  # x/h processed in NCH equal token chunks

_BUILD_CACHE = {}


def _capacity(max_load):
    """Uniform per-core capacity: multiple of NCH*16 so chunks are equal and
    16-aligned."""
    g = NCH * 16
    return max(768, -(-max_load // g) * g)


def _build(C):
    import concourse.mybir as mybir
    import concourse.tile as tile
    from concourse import bacc

    f32 = mybir.dt.float32
    f8 = mybir.dt.float8e4
    f32r = mybir.dt.float32r
    DR = mybir.MatmulPerfMode.DoubleRow

    assert C % (NCH * 16) == 0
    cn = C // NCH
    chunks = [(i * cn, cn) for i in range(NCH)]

    nc = bacc.Bacc("TRN2", target_bir_lowering=False)
    # chunk-major fp8 x (hi/lo packed) so each chunk is one full-rate DMA
    x8 = nc.dram_tensor("x8", (NCH, P, 2, DK, cn), f8, kind="ExternalInput")
    w1 = nc.dram_tensor("w1", (P, HT, 2, DK, P), f8, kind="ExternalInput")
    w2 = nc.dram_tensor("w2", (P, OT, 2, HK, P), f8, kind="ExternalInput")
    b1s = nc.dram_tensor("b1s", (P, HT), f32, kind="ExternalInput")  # 2^HS*b1
    b2s = nc.dram_tensor("b2s", (P, OT), f32, kind="ExternalInput")
    yT = nc.dram_tensor("yT", (O, C), f32, kind="ExternalOutput")

    with tile.TileContext(nc) as tc:
        with (
            tc.tile_pool(name="const", bufs=1) as constp,
            tc.tile_pool(name="main", bufs=1) as mainp,
            tc.tile_pool(name="tmp", bufs=4) as tmpp,
            tc.tile_pool(name="yp", bufs=3) as yp,
            tc.tile_pool(name="ps", bufs=7, space="PSUM") as psp,
            tc.tile_pool(name="warmp", bufs=1, space="PSUM") as warmp,
        ):
            # PE warm-up: dummy f32r matmuls keep the PE busy through the
            # initial DMA window so the HAM clock is fully ramped (3us of
            # continuous execution) when real work arrives.
            warm_w = constp.tile([P, P], f32r, name="warm_w")
            warm_x = constp.tile([P, 256], f32r, name="warm_x")
            nc.vector.memset(warm_w[:].bitcast(mybir.dt.uint32), 0)
            nc.vector.memset(warm_x[:].bitcast(mybir.dt.uint32), 0)
            warm_ps = warmp.tile([P, 256], f32, name="warm_ps")
            for _ in range(16):
                nc.tensor.matmul(
                    warm_ps[:, :], warm_w[:, :], warm_x[:, :],
                    start=True, stop=True,
                )

            # Weights + x all on the SP queue in exact consumption order:
            # the cost model's DMA device is serialized FIFO, so one ordered
            # stream guarantees x chunks are never stuck behind weights.
            w1_sb = mainp.tile([P, HT, 2, DK, P], f8)
            w2_sb = mainp.tile([P, OT, 2, HK, P], f8)
            x_sb = mainp.tile([P, NCH, 2, DK, cn], f8)
            nc.sync.dma_start(w1_sb[:, 0:2], w1[:, 0:2])
            nc.sync.dma_start(x_sb[:, 0], x8[0])
            nc.sync.dma_start(w1_sb[:, 2:8], w1[:, 2:8])
            if NCH > 1:
                nc.sync.dma_start(x_sb[:, 1], x8[1])
            nc.sync.dma_start(w1_sb[:, 8:16], w1[:, 8:16])
            for i in range(2, NCH):
                nc.sync.dma_start(x_sb[:, i], x8[i])
            nc.sync.dma_start(w2_sb[:, 0:4], w2[:, 0:4])
            nc.sync.dma_start(w2_sb[:, 4:8], w2[:, 4:8])

            b1_sb = constp.tile([P, HT], f32)
            nc.gpsimd.dma_start(b1_sb[:], b1s[:])
            b2_sb = constp.tile([P, OT], f32)
            nc.gpsimd.dma_start(b2_sb[:], b2s[:])

            hh_sb = mainp.tile([P, HT, C], f8)
            hl_sb = mainp.tile([P, HT, C], f8)

            # Phase 1: h[ht] = relu(2^-1 * ps + 2^HS*b1),  ps = 2^6 x@W1
            # chunk-outer: chunk 0 feeds the first 16 groups, so only the
            # first x DMA gates startup.
            for ci, (c0, cnn) in enumerate(chunks):
                for ht in range(HT):
                    ps = psp.tile(
                        [P, 512], f32, tag="ps", name=f"ps1_{ht}_{ci}"
                    )[:, :cnn]
                    n = 0
                    for wi, xi in ((0, 0), (1, 0), (0, 1)):
                        for j in range(DK // 2):
                            nc.tensor.matmul(
                                ps,
                                w1_sb[:, ht, wi, 2 * j : 2 * j + 2, :],
                                x_sb[:, ci, xi, 2 * j : 2 * j + 2, :],
                                start=(n == 0),
                                stop=(n == 3 * DK // 2 - 1),
                                perf_mode=DR,
                            )
                            n += 1
                    tmp = tmpp.tile([P, 512], f32, tag="tmp", name=f"t_{ht}_{ci}")[
                        :, :cnn
                    ]
                    nc.scalar.activation(
                        tmp,
                        ps,
                        mybir.ActivationFunctionType.Relu,
                        bias=b1_sb[:, ht : ht + 1],
                        scale=float(2.0 ** (HS - SW)),
                    )
                    nc.vector.tensor_copy(hh_sb[:, ht, c0 : c0 + cnn], tmp)
                    nc.vector.tensor_tensor(
                        hl_sb[:, ht, c0 : c0 + cnn],
                        tmp,
                        hh_sb[:, ht, c0 : c0 + cnn],
                        mybir.AluOpType.subtract,
                    )

            # Phase 2: y[ot] = 2^-(HS+SW) * ps2 + b2,  ps2 = 2^(HS+SW) h@W2
            for ot in range(OT):
                y_sb = yp.tile([P, C], f32, tag="y", name=f"y_{ot}")
                for ci, (c0, cnn) in enumerate(reversed(chunks)):
                    ps = psp.tile(
                        [P, 512], f32, tag="ps", name=f"ps2_{ot}_{ci}"
                    )[:, :cnn]
                    n = 0
                    for wi, hsb in ((0, hh_sb), (1, hh_sb), (0, hl_sb)):
                        for j in range(HK // 2):
                            nc.tensor.matmul(
                                ps,
                                w2_sb[:, ot, wi, 2 * j : 2 * j + 2, :],
                                hsb[:, 2 * j : 2 * j + 2, c0 : c0 + cnn],
                                start=(n == 0),
                                stop=(n == 3 * HK // 2 - 1),
                                perf_mode=DR,
                            )
                            n += 1
                    nc.scalar.activation(
                        y_sb[:, c0 : c0 + cnn],
                        ps,
                        mybir.ActivationFunctionType.Identity,
                        bias=b2_sb[:, ot : ot + 1],
                        scale=float(2.0 ** (-HS - SW)),
                    )
                    if ot == OT - 1:
                        # tail: stagger the final output in per-chunk pieces
                        # via the SWDGE path (descriptors pre-generated, no
                        # HWDGE/dge-delay on the critical tail chain)
                        nc.gpsimd.dma_start(
                            yT[ot * P : (ot + 1) * P, c0 : c0 + cnn],
                            y_sb[:, c0 : c0 + cnn],
                        )
                if ot < OT - 1:
                    nc.scalar.dma_start(yT[ot * P : (ot + 1) * P, :], y_sb[:])

    nc.compile()
    return nc


LAST_BUILD_KEY = None


def _get_built(C):
    global LAST_BUILD_KEY
    key = (C,)
    if key not in _BUILD_CACHE:
        _BUILD_CACHE[key] = _build(C)
    LAST_BUILD_KEY = key
    return _BUILD_CACHE[key]


_RUNNER_CACHE = {}
_WEIGHT_CACHE = {}


def _get_runner(C):
    """Reusable jitted SPMD executable for the bass program (compile once)."""
    key = (C,)
    if key in _RUNNER_CACHE:
        return _RUNNER_CACHE[key]

    import jax
    import concourse.mybir as mybir
    from concourse import bass2jax
    from jax.experimental.shard_map import shard_map
    from jax.sharding import Mesh, NamedSharding, PartitionSpec

    nc = _get_built(C)
    bass2jax.install_neuronx_cc_hook()

    partition_name = (
        nc.partition_id_tensor.name if nc.partition_id_tensor else None
    )
    in_names, out_names, out_avals = [], [], []
    for alloc in nc.m.functions[0].allocations:
        if not isinstance(alloc, mybir.MemoryLocationSet):
            continue
        name = alloc.memorylocations[0].name
        if alloc.kind == "ExternalInput":
            if name != partition_name:
                in_names.append(name)
        elif alloc.kind == "ExternalOutput":
            out_names.append(name)
            out_avals.append(
                jax.core.ShapedArray(
                    tuple(alloc.tensor_shape), mybir.dt.np(alloc.dtype)
                )
            )
    all_names = list(in_names) + list(out_names) + (
        [partition_name] if partition_name else []
    )

    def _body(*args):
        operands = list(args)
        if partition_name is not None:
            operands.append(bass2jax.partition_id_tensor())
        outs = bass2jax._bass_exec_p.bind(
            *operands,
            out_avals=tuple(out_avals),
            in_names=tuple(all_names),
            out_names=tuple(out_names),
            lowering_input_output_aliases=(),
            sim_require_finite=True,
            sim_require_nnan=True,
            nc=nc,
        )
        return tuple(outs)

    devices = jax.devices()[:E]
    mesh = Mesh(np.asarray(devices), ("core",))
    n_io = len(in_names) + len(out_names)
    fn = jax.jit(
        shard_map(
            _body,
            mesh=mesh,
            in_specs=(PartitionSpec("core"),) * n_io,
            out_specs=(PartitionSpec("core"),) * len(out_names),
            check_rep=False,
        ),
        keep_unused=True,
    )
    sharding = NamedSharding(mesh, PartitionSpec("core"))
    # Zero-filled output parameter buffers, device-resident. Not donated: the
    # kernel writes every element of its outputs, so reuse across calls is
    # safe.
    zeros = [
        jax.device_put(
            np.zeros((E * av.shape[0], *av.shape[1:]), av.dtype), sharding
        )
        for av in out_avals
    ]
    runner = {
        "fn": fn,
        "in_names": in_names,
        "out_names": out_names,
        "sharding": sharding,
        "zeros": zeros,
    }
    _RUNNER_CACHE[key] = runner
    return runner


def _weights_fingerprint(arrays):
    import hashlib

    h = hashlib.sha1()
    for k in sorted(arrays):
        a = np.ascontiguousarray(arrays[k])
        h.update(k.encode())
        h.update(str(a.shape).encode())
        flat = a.view(np.uint8).reshape(-1)
        h.update(flat[:: max(1, flat.size // 262144)].tobytes())  # ~256KB sample
        h.update(flat[-4096:].tobytes())
    return h.hexdigest()


def _device_weights(runner, key, arrays):
    """device_put the per-core-stacked weight arrays once, keyed by content."""
    import jax

    fp = (key, _weights_fingerprint(arrays))
    if fp not in _WEIGHT_CACHE:
        _WEIGHT_CACHE.clear()  # keep at most one weight set resident
        _WEIGHT_CACHE[fp] = {
            k: jax.device_put(v, runner["sharding"]) for k, v in arrays.items()
        }
    return _WEIGHT_CACHE[fp]


def _route(x, Wg, bg):
    """Host gating in float64; returns per-expert token ids and gate weights."""
    logits = x.astype(np.float64) @ Wg.astype(np.float64) + bg.astype(np.float64)
    order = np.argsort(-logits, axis=1, kind="stable")
    top2 = order[:, :TOPK]  # [T, 2]
    v = np.take_along_axis(logits, top2, axis=1)
    ex = np.exp(v - v.max(axis=1, keepdims=True))
    g = (ex / ex.sum(axis=1, keepdims=True)).astype(np.float32)  # [T, 2]
    ids, gates = [], []
    for e in range(E):
        sel = top2 == e  # [T, 2]
        te = np.where(sel.any(axis=1))[0]
        ge = np.where(sel[te, 0], g[te, 0], g[te, 1])
        ids.append(te)
        gates.append(ge.astype(np.float32))
    return ids, gates


def _f8():
    import ml_dtypes

    return np.dtype(ml_dtypes.float8_e4m3)


def _split_f8(a):
    """Return (hi, lo) e4m3 arrays with hi + lo ~= a."""
    f8 = _f8()
    hi = a.astype(f8)
    lo = (a - hi.astype(np.float32)).astype(f8)
    return hi, lo


def _prep_weights(W1, b1, W2, b2):
    """Quantize + lay out weights for the kernel, stacked per core.

    w1 tile layout: [p, ht, dk, m] = W1s[dk*128+p, ht*128+m]
    w2 tile layout: [p, ot, hk, m] = W2s[hk*128+p, ot*128+m]
    """
    s = np.float32(2.0**SW)
    arrs = {}
    # [E, D, H] -> [E, dk, p, ht, m] -> [E*p, ht, dk, m]
    W1s = (W1 * s).reshape(E, DK, P, HT, P)
    W2s = (W2 * s).reshape(E, HK, P, OT, P)
    for name, Ws in (("w1", W1s), ("w2", W2s)):
        hi, lo = _split_f8(Ws.astype(np.float32))
        # [e, k_tiles, p, out_tiles, m] -> [e, p, out_tiles, {hi,lo}, k_tiles, m]
        t = np.stack(
            (hi.transpose(0, 2, 3, 1, 4), lo.transpose(0, 2, 3, 1, 4)), axis=3
        )
        arrs[name] = np.ascontiguousarray(
            t.reshape(E * P, t.shape[2], 2, t.shape[4], P)
        )
    arrs["b1s"] = np.ascontiguousarray(
        (b1 * np.float32(2.0**HS)).reshape(E, HT, P).transpose(0, 2, 1).reshape(E * P, HT)
    ).astype(np.float32)
    arrs["b2s"] = np.ascontiguousarray(
        b2.reshape(E, OT, P).transpose(0, 2, 1).reshape(E * P, OT)
    ).astype(np.float32)
    return arrs


def _is_axon():
    try:
        from concourse._compat import axon_active

        return bool(axon_active())
    except Exception:  # noqa: BLE001
        return False


def _build_x_global(C, ids, x):
    """Chunk-major fp8 hi/lo-packed x dispatch array, stacked per core.

    Returns x8_g of shape [E*NCH, P, 2, DK, cn]; core e's slice is
    [e*NCH:(e+1)*NCH] with layout [chunk, p, hi/lo, dk, c].
    """
    cn = C // NCH
    f8 = _f8()
    x8_g = np.zeros((E, NCH, P, 2, DK, cn), f8)
    for e in range(E):
        te = ids[e]
        if len(te) == 0:
            continue
        xt = np.zeros((C, DK, P), np.float32)
        xt[: len(te)] = x[te].reshape(len(te), DK, P)
        hi, lo = _split_f8(xt)
        # [C, dk, p] -> [nch, cn, dk, p] -> [nch, p, dk, cn]
        x8_g[e, :, :, 0] = hi.reshape(NCH, cn, DK, P).transpose(0, 3, 2, 1)
        x8_g[e, :, :, 1] = lo.reshape(NCH, cn, DK, P).transpose(0, 3, 2, 1)
    return np.ascontiguousarray(x8_g.reshape(E * NCH, P, 2, DK, cn))


def _run_axon(C, ids, x, warrs):
    """Fast path: cached jitted SPMD executable, device-resident weights."""
    import jax

    runner = _get_runner(C)
    dev_w = _device_weights(runner, (C,), warrs)

    x8_g = _build_x_global(C, ids, x)
    x8_dev = jax.device_put(x8_g, runner["sharding"])

    operands = []
    for name in runner["in_names"]:
        if name == "x8":
            operands.append(x8_dev)
        else:
            operands.append(dev_w[name])
    operands.extend(runner["zeros"])
    outs = runner["fn"](*operands)
    return np.asarray(outs[runner["out_names"].index("yT")])  # [E*O, C]


def _run_native(C, ids, x, warrs):
    """Fallback for non-axon environments: bass_utils native NRT runner."""
    from concourse.bass_utils import run_bass_kernel_spmd

    nc = _get_built(C)
    x8_g = _build_x_global(C, ids, x)
    in_maps = []
    for e in range(E):
        m = {
            "x8": np.ascontiguousarray(x8_g[e * NCH : (e + 1) * NCH]),
        }
        for k, v in warrs.items():
            m[k] = np.ascontiguousarray(v[e * P : (e + 1) * P])
        in_maps.append(m)
    res = run_bass_kernel_spmd(nc, in_maps, core_ids=list(range(E)))
    return np.concatenate([res.results[e]["yT"] for e in range(E)], axis=0)


# Above this capacity the working set (x + h + y tiles at current pool
# depths) overflows SBUF; heavier routing skew runs as multiple batches.
_MAX_C = 1920

FALLBACK_USED = False  # set when the numpy emergency path ran (device down)


def _run_device(C, bids, x, warrs, W1, b1, W2, b2):
    """Run the bass kernel on the 8 cores, with one retry after a device
    error and a loud numpy fallback if the accelerator is unrecoverable."""
    for attempt in range(2):
        try:
            if _is_axon():
                return _run_axon(C, bids, x, warrs)
            return _run_native(C, bids, x, warrs)
        except Exception as ex:  # noqa: BLE001
            print(
                f"kernel: device run failed (attempt {attempt}): "
                f"{type(ex).__name__}: {str(ex)[:200]}",
                flush=True,
            )
            # Device arrays / executables may be poisoned; rebuild them.
            _RUNNER_CACHE.clear()
            _WEIGHT_CACHE.clear()
            try:
                import jax

                jax.clear_caches()
            except Exception:  # noqa: BLE001
                pass
    global FALLBACK_USED
    FALLBACK_USED = True
    print(
        "kernel: WARNING - accelerator unavailable after retries; "
        "computing this batch on the host (numpy) so the result is correct",
        flush=True,
    )
    yT_g = np.zeros((E * O, C), np.float32)
    for e in range(E):
        te = bids[e]
        if len(te) == 0:
            continue
        h = np.maximum(x[te] @ W1[e] + b1[e], 0.0)
        yT_g[e * O : (e + 1) * O, : len(te)] = (h @ W2[e] + b2[e]).T
    return yT_g


def kernel(x, Wg, bg, W1, b1, W2, b2):
    x = np.ascontiguousarray(np.asarray(x, np.float32))
    Wg = np.asarray(Wg, np.float32)
    bg = np.asarray(bg, np.float32)
    W1 = np.ascontiguousarray(np.asarray(W1, np.float32))
    b1 = np.ascontiguousarray(np.asarray(b1, np.float32))
    W2 = np.ascontiguousarray(np.asarray(W2, np.float32))
    b2 = np.ascontiguousarray(np.asarray(b2, np.float32))

    assert x.shape[1] == D and Wg.shape == (D, E)
    assert W1.shape == (E, D, H) and W2.shape == (E, H, O)

    ids, gates = _route(x, Wg, bg)

    warrs = _prep_weights(W1, b1, W2, b2)

    out = np.zeros((x.shape[0], O), np.float32)
    max_load = max(len(te) for te in ids)
    n_batches = -(-max_load // _MAX_C)
    for b in range(n_batches):
        bids = [te[b * _MAX_C : (b + 1) * _MAX_C] for te in ids]
        C = _capacity(max(len(te) for te in bids))
        yT_g = _run_device(C, bids, x, warrs, W1, b1, W2, b2)
        for e in range(E):
            te = bids[e]
            ge = gates[e][b * _MAX_C : (b + 1) * _MAX_C]
            ye = yT_g[e * O : e * O + O, : len(te)].T  # [n_e, O]
            out[te] += ge[:, None] * ye
    return out


# revision 9
# speedup vs baseline: 1.0860x; 1.0246x over previous
"""MoE (top-2 routing, 8 experts) Trainium2 kernel — fp8 DoubleRow version.

Strategy (expert-parallel, matches the sharding hint):
  - Gating (x @ Wg + bg, top-2, softmax) is computed on the host in float64.
    The top-2/3rd logit gap for these inputs is >=1.6e-5, far above fp32
    rounding noise, so the host selection matches the fp32 reference exactly.
  - Tokens are dispatched by expert id: core e receives the tokens routed to
    expert e (padded to a uniform capacity C), plus expert e's weights.
  - Each core runs a Bass/Tile kernel computing
        yT = (relu(x @ W1 + b1) @ W2 + b2)^T      (shape [O, C])
  - The host combines: out[t] = sum_k gate[t,k] * y_{expert_k(t)}[t].

Compute scheme: fp8e4m3 hi/lo split with DoubleRow matmuls.
  Every operand A (x, W1, h, W2) is represented as A_hi + A_lo, both e4m3
  (A_lo = e4m3(A - A_hi)), with weights pre-scaled by 2^6 and h stored at
  2^HS so everything sits in e4m3's normal range. Each matmul product is
  computed in three passes accumulated in PSUM:
        A@B ~= A_hi@B_hi + A_hi@B_lo + A_lo@B_hi
  (the dropped lo@lo term is ~1e-4 relative). DoubleRow contracts 2 k-tiles
  (256) per instruction at 0.5 cycles/row, so the 3-pass scheme costs 0.75x
  a single bf16 pass while keeping ~bf16 accuracy (measured 2e-3 end to end).
  All scales are powers of two folded into the ACT-engine epilogues (relu is
  positively homogeneous), so no extra scaling ops are needed.
"""

import numpy as np

T, D, H, O, E, TOPK = 4096, 1024, 2048, 1024, 8, 2
P = 128
DK, HK, HT, OT = D // P, H // P, H // P, O // P

SW = 6   # W1/W2 stored as e4m3(W * 2^SW)
HS = 5   # h stored as 2^HS * relu(x@W1 + b1)  (max |h|*2^5 ~ 96 << 240)

NCH = 3  # x/h processed in NCH equal token chunks

_BUILD_CACHE = {}


def _capacity(max_load):
    """Uniform per-core capacity: multiple of NCH*16 so chunks are equal and
    16-aligned."""
    g = NCH * 16
    return max(768, -(-max_load // g) * g)


def _build(C):
    import concourse.mybir as mybir
    import concourse.tile as tile
    from concourse import bacc

    f32 = mybir.dt.float32
    f8 = mybir.dt.float8e4
    f32r = mybir.dt.float32r
    DR = mybir.MatmulPerfMode.DoubleRow

    assert C % (NCH * 16) == 0
    cn = C // NCH
    chunks = [(i * cn, cn) for i in range(NCH)]

    nc = bacc.Bacc("TRN2", target_bir_lowering=False)
    # chunk-major fp8 x (hi/lo packed) so each chunk is one full-rate DMA
    x8 = nc.dram_tensor("x8", (NCH, P, 2, DK, cn), f8, kind="ExternalInput")
    w1 = nc.dram_tensor("w1", (P, HT, 2, DK, P), f8, kind="ExternalInput")
    w2 = nc.dram_tensor("w2", (P, OT, 2, HK, P), f8, kind="ExternalInput")
    b1s = nc.dram_tensor("b1s", (P, HT), f32, kind="ExternalInput")  # 2^HS*b1
    b2s = nc.dram_tensor("b2s", (P, OT), f32, kind="ExternalInput")
    yT = nc.dram_tensor("yT", (O, C), f32, kind="ExternalOutput")

    with tile.TileContext(nc) as tc:
        with (
            tc.tile_pool(name="const", bufs=1) as constp,
            tc.tile_pool(name="main", bufs=1) as mainp,
            tc.tile_pool(name="tmp", bufs=4) as tmpp,
            tc.tile_pool(name="yp", bufs=3) as yp,
            tc.tile_pool(name="ps", bufs=7, space="PSUM") as psp,
            tc.tile_pool(name="warmp", bufs=1, space="PSUM") as warmp,
        ):
            # PE warm-up: dummy f32r matmuls keep the PE busy through the
            # initial DMA window so the HAM clock is fully ramped (3us of
            # continuous execution) when real work arrives.
            warm_w = constp.tile([P, P], f32r, name="warm_w")
            warm_x = constp.tile([P, 256], f32r, name="warm_x")
            nc.vector.memset(warm_w[:].bitcast(mybir.dt.uint32), 0)
            nc.vector.memset(warm_x[:].bitcast(mybir.dt.uint32), 0)
            warm_ps = warmp.tile([P, 256], f32, name="warm_ps")
            for _ in range(18):
                nc.tensor.matmul(
                    warm_ps[:, :], warm_w[:, :], warm_x[:, :],
                    start=True, stop=True,
                )

            # Weights + x all on the SP queue in exact consumption order:
            # the cost model's DMA device is serialized FIFO, so one ordered
            # stream guarantees x chunks are never stuck behind weights.
            w1_sb = mainp.tile([P, HT, 2, DK, P], f8)
            w2_sb = mainp.tile([P, OT, 2, HK, P], f8)
            x_sb = mainp.tile([P, NCH, 2, DK, cn], f8)
            # Interleave x chunks between w1 ht-pair slices so neither
            # stream starves the in-order PE consumption (c0 sweeps ht0..15
            # at ~1.4us per pair; each pair transfer is ~1.5us).
            nc.sync.dma_start(x_sb[:, 0], x8[0])
            nc.sync.dma_start(w1_sb[:, 0:2], w1[:, 0:2])
            nc.sync.dma_start(w1_sb[:, 2:4], w1[:, 2:4])
            if NCH > 1:
                nc.sync.dma_start(x_sb[:, 1], x8[1])
            nc.sync.dma_start(w1_sb[:, 4:6], w1[:, 4:6])
            nc.sync.dma_start(w1_sb[:, 6:8], w1[:, 6:8])
            nc.sync.dma_start(w1_sb[:, 8:10], w1[:, 8:10])
            for i in range(2, NCH):
                nc.sync.dma_start(x_sb[:, i], x8[i])
            nc.sync.dma_start(w1_sb[:, 10:12], w1[:, 10:12])
            nc.sync.dma_start(w1_sb[:, 12:14], w1[:, 12:14])
            nc.sync.dma_start(w1_sb[:, 14:16], w1[:, 14:16])
            for a in range(0, OT, 2):
                nc.sync.dma_start(w2_sb[:, a : a + 2], w2[:, a : a + 2])

            b1_sb = constp.tile([P, HT], f32)
            nc.gpsimd.dma_start(b1_sb[:], b1s[:])
            b2_sb = constp.tile([P, OT], f32)
            nc.gpsimd.dma_start(b2_sb[:], b2s[:])

            hh_sb = mainp.tile([P, HT, C], f8)
            hl_sb = mainp.tile([P, HT, C], f8)

            # Phase 1: h[ht] = relu(2^-1 * ps + 2^HS*b1),  ps = 2^6 x@W1
            # chunk-outer: chunk 0 feeds the first 16 groups, so only the
            # first x DMA gates startup.
            for ci, (c0, cnn) in enumerate(chunks):
                for ht in range(HT):
                    ps = psp.tile(
                        [P, 512], f32, tag="ps", name=f"ps1_{ht}_{ci}"
                    )[:, :cnn]
                    n = 0
                    for wi, xi in ((0, 0), (1, 0), (0, 1)):
                        for j in range(DK // 2):
                            nc.tensor.matmul(
                                ps,
                                w1_sb[:, ht, wi, 2 * j : 2 * j + 2, :],
                                x_sb[:, ci, xi, 2 * j : 2 * j + 2, :],
                                start=(n == 0),
                                stop=(n == 3 * DK // 2 - 1),
                                perf_mode=DR,
                            )
                            n += 1
                    tmp = tmpp.tile([P, 512], f32, tag="tmp", name=f"t_{ht}_{ci}")[
                        :, :cnn
                    ]
                    nc.scalar.activation(
                        tmp,
                        ps,
                        mybir.ActivationFunctionType.Relu,
                        bias=b1_sb[:, ht : ht + 1],
                        scale=float(2.0 ** (HS - SW)),
                    )
                    nc.vector.tensor_copy(hh_sb[:, ht, c0 : c0 + cnn], tmp)
                    nc.vector.tensor_tensor(
                        hl_sb[:, ht, c0 : c0 + cnn],
                        tmp,
                        hh_sb[:, ht, c0 : c0 + cnn],
                        mybir.AluOpType.subtract,
                    )

            # Phase 2: y[ot] = 2^-(HS+SW) * ps2 + b2,  ps2 = 2^(HS+SW) h@W2
            for ot in range(OT):
                y_sb = yp.tile([P, C], f32, tag="y", name=f"y_{ot}")
                for ci, (c0, cnn) in enumerate(reversed(chunks)):
                    ps = psp.tile(
                        [P, 512], f32, tag="ps", name=f"ps2_{ot}_{ci}"
                    )[:, :cnn]
                    n = 0
                    for wi, hsb in ((0, hh_sb), (1, hh_sb), (0, hl_sb)):
                        for j in range(HK // 2):
                            nc.tensor.matmul(
                                ps,
                                w2_sb[:, ot, wi, 2 * j : 2 * j + 2, :],
                                hsb[:, 2 * j : 2 * j + 2, c0 : c0 + cnn],
                                start=(n == 0),
                                stop=(n == 3 * HK // 2 - 1),
                                perf_mode=DR,
                            )
                            n += 1
                    nc.scalar.activation(
                        y_sb[:, c0 : c0 + cnn],
                        ps,
                        mybir.ActivationFunctionType.Identity,
                        bias=b2_sb[:, ot : ot + 1],
                        scale=float(2.0 ** (-HS - SW)),
                    )
                    if ot == OT - 1:
                        # tail: stagger the final output per chunk; the very
                        # last chunk goes in two halves so the final DMA (and
                        # its post-data latency) covers only half a chunk
                        if ci == NCH - 1:
                            half = cnn // 2
                            for h0, hn in ((0, half), (half, cnn - half)):
                                nc.scalar.dma_start(
                                    yT[
                                        ot * P : (ot + 1) * P,
                                        c0 + h0 : c0 + h0 + hn,
                                    ],
                                    y_sb[:, c0 + h0 : c0 + h0 + hn],
                                )
                        else:
                            nc.scalar.dma_start(
                                yT[ot * P : (ot + 1) * P, c0 : c0 + cnn],
                                y_sb[:, c0 : c0 + cnn],
                            )
                if ot < OT - 1:
                    nc.scalar.dma_start(yT[ot * P : (ot + 1) * P, :], y_sb[:])

    nc.compile()
    return nc


LAST_BUILD_KEY = None


def _get_built(C):
    global LAST_BUILD_KEY
    key = (C,)
    if key not in _BUILD_CACHE:
        _BUILD_CACHE[key] = _build(C)
    LAST_BUILD_KEY = key
    return _BUILD_CACHE[key]


_RUNNER_CACHE = {}
_WEIGHT_CACHE = {}


def _get_runner(C):
    """Reusable jitted SPMD executable for the bass program (compile once)."""
    key = (C,)
    if key in _RUNNER_CACHE:
        return _RUNNER_CACHE[key]

    import jax
    import concourse.mybir as mybir
    from concourse import bass2jax
    from jax.experimental.shard_map import shard_map
    from jax.sharding import Mesh, NamedSharding, PartitionSpec

    nc = _get_built(C)
    bass2jax.install_neuronx_cc_hook()

    partition_name = (
        nc.partition_id_tensor.name if nc.partition_id_tensor else None
    )
    in_names, out_names, out_avals = [], [], []
    for alloc in nc.m.functions[0].allocations:
        if not isinstance(alloc, mybir.MemoryLocationSet):
            continue
        name = alloc.memorylocations[0].name
        if alloc.kind == "ExternalInput":
            if name != partition_name:
                in_names.append(name)
        elif alloc.kind == "ExternalOutput":
            out_names.append(name)
            out_avals.append(
                jax.core.ShapedArray(
                    tuple(alloc.tensor_shape), mybir.dt.np(alloc.dtype)
                )
            )
    all_names = list(in_names) + list(out_names) + (
        [partition_name] if partition_name else []
    )

    def _body(*args):
        operands = list(args)
        if partition_name is not None:
            operands.append(bass2jax.partition_id_tensor())
        outs = bass2jax._bass_exec_p.bind(
            *operands,
            out_avals=tuple(out_avals),
            in_names=tuple(all_names),
            out_names=tuple(out_names),
            lowering_input_output_aliases=(),
            sim_require_finite=True,
            sim_require_nnan=True,
            nc=nc,
        )
        return tuple(outs)

    devices = jax.devices()[:E]
    mesh = Mesh(np.asarray(devices), ("core",))
    n_io = len(in_names) + len(out_names)
    fn = jax.jit(
        shard_map(
            _body,
            mesh=mesh,
            in_specs=(PartitionSpec("core"),) * n_io,
            out_specs=(PartitionSpec("core"),) * len(out_names),
            check_rep=False,
        ),
        keep_unused=True,
    )
    sharding = NamedSharding(mesh, PartitionSpec("core"))
    # Zero-filled output parameter buffers, device-resident. Not donated: the
    # kernel writes every element of its outputs, so reuse across calls is
    # safe.
    zeros = [
        jax.device_put(
            np.zeros((E * av.shape[0], *av.shape[1:]), av.dtype), sharding
        )
        for av in out_avals
    ]
    runner = {
        "fn": fn,
        "in_names": in_names,
        "out_names": out_names,
        "sharding": sharding,
        "zeros": zeros,
    }
    _RUNNER_CACHE[key] = runner
    return runner


def _weights_fingerprint(arrays):
    import hashlib

    h = hashlib.sha1()
    for k in sorted(arrays):
        a = np.ascontiguousarray(arrays[k])
        h.update(k.encode())
        h.update(str(a.shape).encode())
        flat = a.view(np.uint8).reshape(-1)
        h.update(flat[:: max(1, flat.size // 262144)].tobytes())  # ~256KB sample
        h.update(flat[-4096:].tobytes())
    return h.hexdigest()


def _device_weights(runner, key, arrays):
    """device_put the per-core-stacked weight arrays once, keyed by content."""
    import jax

    fp = (key, _weights_fingerprint(arrays))
    if fp not in _WEIGHT_CACHE:
        _WEIGHT_CACHE.clear()  # keep at most one weight set resident
        _WEIGHT_CACHE[fp] = {
            k: jax.device_put(v, runner["sharding"]) for k, v in arrays.items()
        }
    return _WEIGHT_CACHE[fp]


def _route(x, Wg, bg):
    """Host gating in float64; returns per-expert token ids and gate weights."""
    logits = x.astype(np.float64) @ Wg.astype(np.float64) + bg.astype(np.float64)
    order = np.argsort(-logits, axis=1, kind="stable")
    top2 = order[:, :TOPK]  # [T, 2]
    v = np.take_along_axis(logits, top2, axis=1)
    ex = np.exp(v - v.max(axis=1, keepdims=True))
    g = (ex / ex.sum(axis=1, keepdims=True)).astype(np.float32)  # [T, 2]
    ids, gates = [], []
    for e in range(E):
        sel = top2 == e  # [T, 2]
        te = np.where(sel.any(axis=1))[0]
        ge = np.where(sel[te, 0], g[te, 0], g[te, 1])
        ids.append(te)
        gates.append(ge.astype(np.float32))
    return ids, gates


def _f8():
    import ml_dtypes

    return np.dtype(ml_dtypes.float8_e4m3)


def _split_f8(a):
    """Return (hi, lo) e4m3 arrays with hi + lo ~= a."""
    f8 = _f8()
    hi = a.astype(f8)
    lo = (a - hi.astype(np.float32)).astype(f8)
    return hi, lo


def _prep_weights(W1, b1, W2, b2):
    """Quantize + lay out weights for the kernel, stacked per core.

    w1 tile layout: [p, ht, dk, m] = W1s[dk*128+p, ht*128+m]
    w2 tile layout: [p, ot, hk, m] = W2s[hk*128+p, ot*128+m]
    """
    s = np.float32(2.0**SW)
    arrs = {}
    # [E, D, H] -> [E, dk, p, ht, m] -> [E*p, ht, dk, m]
    W1s = (W1 * s).reshape(E, DK, P, HT, P)
    W2s = (W2 * s).reshape(E, HK, P, OT, P)
    for name, Ws in (("w1", W1s), ("w2", W2s)):
        hi, lo = _split_f8(Ws.astype(np.float32))
        # [e, k_tiles, p, out_tiles, m] -> [e, p, out_tiles, {hi,lo}, k_tiles, m]
        t = np.stack(
            (hi.transpose(0, 2, 3, 1, 4), lo.transpose(0, 2, 3, 1, 4)), axis=3
        )
        arrs[name] = np.ascontiguousarray(
            t.reshape(E * P, t.shape[2], 2, t.shape[4], P)
        )
    arrs["b1s"] = np.ascontiguousarray(
        (b1 * np.float32(2.0**HS)).reshape(E, HT, P).transpose(0, 2, 1).reshape(E * P, HT)
    ).astype(np.float32)
    arrs["b2s"] = np.ascontiguousarray(
        b2.reshape(E, OT, P).transpose(0, 2, 1).reshape(E * P, OT)
    ).astype(np.float32)
    return arrs


def _is_axon():
    try:
        from concourse._compat import axon_active

        return bool(axon_active())
    except Exception:  # noqa: BLE001
        return False


def _build_x_global(C, ids, x):
    """Chunk-major fp8 hi/lo-packed x dispatch array, stacked per core.

    Returns x8_g of shape [E*NCH, P, 2, DK, cn]; core e's slice is
    [e*NCH:(e+1)*NCH] with layout [chunk, p, hi/lo, dk, c].
    """
    cn = C // NCH
    f8 = _f8()
    x8_g = np.zeros((E, NCH, P, 2, DK, cn), f8)
    for e in range(E):
        te = ids[e]
        if len(te) == 0:
            continue
        xt = np.zeros((C, DK, P), np.float32)
        xt[: len(te)] = x[te].reshape(len(te), DK, P)
        hi, lo = _split_f8(xt)
        # [C, dk, p] -> [nch, cn, dk, p] -> [nch, p, dk, cn]
        x8_g[e, :, :, 0] = hi.reshape(NCH, cn, DK, P).transpose(0, 3, 2, 1)
        x8_g[e, :, :, 1] = lo.reshape(NCH, cn, DK, P).transpose(0, 3, 2, 1)
    return np.ascontiguousarray(x8_g.reshape(E * NCH, P, 2, DK, cn))


def _run_axon(C, ids, x, warrs):
    """Fast path: cached jitted SPMD executable, device-resident weights."""
    import jax

    runner = _get_runner(C)
    dev_w = _device_weights(runner, (C,), warrs)

    x8_g = _build_x_global(C, ids, x)
    x8_dev = jax.device_put(x8_g, runner["sharding"])

    operands = []
    for name in runner["in_names"]:
        if name == "x8":
            operands.append(x8_dev)
        else:
            operands.append(dev_w[name])
    operands.extend(runner["zeros"])
    outs = runner["fn"](*operands)
    return np.asarray(outs[runner["out_names"].index("yT")])  # [E*O, C]


def _run_native(C, ids, x, warrs):
    """Fallback for non-axon environments: bass_utils native NRT runner."""
    from concourse.bass_utils import run_bass_kernel_spmd

    nc = _get_built(C)
    x8_g = _build_x_global(C, ids, x)
    in_maps = []
    for e in range(E):
        m = {
            "x8": np.ascontiguousarray(x8_g[e * NCH : (e + 1) * NCH]),
        }
        for k, v in warrs.items():
            m[k] = np.ascontiguousarray(v[e * P : (e + 1) * P])
        in_maps.append(m)
    res = run_bass_kernel_spmd(nc, in_maps, core_ids=list(range(E)))
    return np.concatenate([res.results[e]["yT"] for e in range(E)], axis=0)


# Above this capacity the working set (x + h + y tiles at current pool
# depths) overflows SBUF; heavier routing skew runs as multiple batches.
_MAX_C = 1920

FALLBACK_USED = False  # set when the numpy emergency path ran (device down)


def _run_device(C, bids, x, warrs, W1, b1, W2, b2):
    """Run the bass kernel on the 8 cores, with one retry after a device
    error and a loud numpy fallback if the accelerator is unrecoverable."""
    for attempt in range(2):
        try:
            if _is_axon():
                return _run_axon(C, bids, x, warrs)
            return _run_native(C, bids, x, warrs)
        except Exception as ex:  # noqa: BLE001
            print(
                f"kernel: device run failed (attempt {attempt}): "
                f"{type(ex).__name__}: {str(ex)[:200]}",
                flush=True,
            )
            # Device arrays / executables may be poisoned; rebuild them.
            _RUNNER_CACHE.clear()
            _WEIGHT_CACHE.clear()
            try:
                import jax

                jax.clear_caches()
            except Exception:  # noqa: BLE001
                pass
    global FALLBACK_USED
    FALLBACK_USED = True
    print(
        "kernel: WARNING - accelerator unavailable after retries; "
        "computing this batch on the host (numpy) so the result is correct",
        flush=True,
    )
    yT_g = np.zeros((E * O, C), np.float32)
    for e in range(E):
        te = bids[e]
        if len(te) == 0:
            continue
        h = np.maximum(x[te] @ W1[e] + b1[e], 0.0)
        yT_g[e * O : (e + 1) * O, : len(te)] = (h @ W2[e] + b2[e]).T
    return yT_g


def kernel(x, Wg, bg, W1, b1, W2, b2):
    x = np.ascontiguousarray(np.asarray(x, np.float32))
    Wg = np.asarray(Wg, np.float32)
    bg = np.asarray(bg, np.float32)
    W1 = np.ascontiguousarray(np.asarray(W1, np.float32))
    b1 = np.ascontiguousarray(np.asarray(b1, np.float32))
    W2 = np.ascontiguousarray(np.asarray(W2, np.float32))
    b2 = np.ascontiguousarray(np.asarray(b2, np.float32))

    assert x.shape[1] == D and Wg.shape == (D, E)
    assert W1.shape == (E, D, H) and W2.shape == (E, H, O)

    ids, gates = _route(x, Wg, bg)

    warrs = _prep_weights(W1, b1, W2, b2)

    out = np.zeros((x.shape[0], O), np.float32)
    max_load = max(len(te) for te in ids)
    n_batches = -(-max_load // _MAX_C)
    for b in range(n_batches):
        bids = [te[b * _MAX_C : (b + 1) * _MAX_C] for te in ids]
        C = _capacity(max(len(te) for te in bids))
        yT_g = _run_device(C, bids, x, warrs, W1, b1, W2, b2)
        for e in range(E):
            te = bids[e]
            ge = gates[e][b * _MAX_C : (b + 1) * _MAX_C]
            ye = yT_g[e * O : e * O + O, : len(te)].T  # [n_e, O]
            out[te] += ge[:, None] * ye
    return out


# revision 10
# speedup vs baseline: 1.1127x; 1.0246x over previous
"""MoE (top-2 routing, 8 experts) Trainium2 kernel — fp8 DoubleRow version.

Strategy (expert-parallel, matches the sharding hint):
  - Gating (x @ Wg + bg, top-2, softmax) is computed on the host in float64.
    The top-2/3rd logit gap for these inputs is >=1.6e-5, far above fp32
    rounding noise, so the host selection matches the fp32 reference exactly.
  - Tokens are dispatched by expert id: core e receives the tokens routed to
    expert e (padded to a uniform capacity C), plus expert e's weights.
  - Each core runs a Bass/Tile kernel computing
        yT = (relu(x @ W1 + b1) @ W2 + b2)^T      (shape [O, C])
  - The host combines: out[t] = sum_k gate[t,k] * y_{expert_k(t)}[t].

Compute scheme: fp8e4m3 hi/lo split with DoubleRow matmuls.
  Every operand A (x, W1, h, W2) is represented as A_hi + A_lo, both e4m3
  (A_lo = e4m3(A - A_hi)), with weights pre-scaled by 2^6 and h stored at
  2^HS so everything sits in e4m3's normal range. Each matmul product is
  computed in three passes accumulated in PSUM:
        A@B ~= A_hi@B_hi + A_hi@B_lo + A_lo@B_hi
  (the dropped lo@lo term is ~1e-4 relative). DoubleRow contracts 2 k-tiles
  (256) per instruction at 0.5 cycles/row, so the 3-pass scheme costs 0.75x
  a single bf16 pass while keeping ~bf16 accuracy (measured 2e-3 end to end).
  All scales are powers of two folded into the ACT-engine epilogues (relu is
  positively homogeneous), so no extra scaling ops are needed.
"""

import numpy as np

T, D, H, O, E, TOPK = 4096, 1024, 2048, 1024, 8, 2
P = 128
DK, HK, HT, OT = D // P, H // P, H // P, O // P

SW = 6   # W1/W2 stored as e4m3(W * 2^SW)
HS = 5   # h stored as 2^HS * relu(x@W1 + b1)  (max |h|*2^5 ~ 96 << 240)

NCH = 3  # x/h processed in NCH equal token chunks

_BUILD_CACHE = {}


def _capacity(max_load):
    """Uniform per-core capacity: multiple of NCH*16 so chunks are equal and
    16-aligned."""
    g = NCH * 16
    return max(768, -(-max_load // g) * g)


def _build(C):
    import concourse.mybir as mybir
    import concourse.tile as tile
    from concourse import bacc

    f32 = mybir.dt.float32
    f8 = mybir.dt.float8e4
    f32r = mybir.dt.float32r
    DR = mybir.MatmulPerfMode.DoubleRow

    assert C % (NCH * 16) == 0
    cn = C // NCH
    chunks = [(i * cn, cn) for i in range(NCH)]

    nc = bacc.Bacc("TRN2", target_bir_lowering=False)
    # chunk-major fp8 x (hi/lo packed) so each chunk is one full-rate DMA
    x8 = nc.dram_tensor("x8", (NCH, P, 2, DK, cn), f8, kind="ExternalInput")
    w1 = nc.dram_tensor("w1", (P, HT, 2, DK, P), f8, kind="ExternalInput")
    w2 = nc.dram_tensor("w2", (P, OT, 2, HK, P), f8, kind="ExternalInput")
    b1s = nc.dram_tensor("b1s", (P, HT), f32, kind="ExternalInput")  # 2^HS*b1
    b2s = nc.dram_tensor("b2s", (P, OT), f32, kind="ExternalInput")
    yT = nc.dram_tensor("yT", (O, C), f32, kind="ExternalOutput")

    with tile.TileContext(nc) as tc:
        with (
            tc.tile_pool(name="const", bufs=1) as constp,
            tc.tile_pool(name="main", bufs=1) as mainp,
            tc.tile_pool(name="tmp", bufs=4) as tmpp,
            tc.tile_pool(name="yp", bufs=3) as yp,
            tc.tile_pool(name="ps", bufs=7, space="PSUM") as psp,
            tc.tile_pool(name="warmp", bufs=1, space="PSUM") as warmp,
        ):
            # PE warm-up: dummy f32r matmuls keep the PE busy through the
            # initial DMA window so the HAM clock is fully ramped (3us of
            # continuous execution) when real work arrives.
            warm_x = constp.tile([P, 256], f32r, name="warm_x")
            nc.vector.memset(warm_x[:].bitcast(mybir.dt.uint32), 0)
            warm_ps = warmp.tile([P, 256], f32, name="warm_ps")
            for _ in range(18):
                nc.tensor.matmul(
                    warm_ps[:, :], warm_x[:, :128], warm_x[:, :],
                    start=True, stop=True,
                )

            # x0 goes via the gpsimd SWDGE path (lowest launch latency:
            # Pool is otherwise idle, descriptor gen starts right after the
            # preamble). Everything else streams on the SP queue in exact
            # consumption order; the cost model's DMA device is serialized
            # FIFO, so one ordered stream keeps supply aligned with the
            # in-order PE demand.
            w1_sb = mainp.tile([P, HT, 2, DK, P], f8)
            w2_sb = mainp.tile([P, OT, 2, HK, P], f8)
            x_sb = mainp.tile([P, NCH, 2, DK, cn], f8)
            nc.gpsimd.dma_start(x_sb[:, 0], x8[0])
            b1_sb = constp.tile([P, HT], f32)
            nc.gpsimd.dma_start(b1_sb[:], b1s[:])
            b2_sb = constp.tile([P, OT], f32)
            nc.gpsimd.dma_start(b2_sb[:], b2s[:])

            for a in range(0, 8, 2):
                nc.sync.dma_start(w1_sb[:, a : a + 2], w1[:, a : a + 2])
            if NCH > 1:
                nc.sync.dma_start(x_sb[:, 1], x8[1])
            for a in range(8, HT, 2):
                nc.sync.dma_start(w1_sb[:, a : a + 2], w1[:, a : a + 2])
            for i in range(2, NCH):
                nc.sync.dma_start(x_sb[:, i], x8[i])
            for a in range(0, OT, 2):
                nc.sync.dma_start(w2_sb[:, a : a + 2], w2[:, a : a + 2])

            hh_sb = mainp.tile([P, HT, C], f8)
            hl_sb = mainp.tile([P, HT, C], f8)

            # Phase 1: h[ht] = relu(2^-1 * ps + 2^HS*b1),  ps = 2^6 x@W1
            # Segment order interleaves ht-halves of chunks 0/1 so the early
            # w1 demand rate is half of the c0-only sweep, matching the
            # serialized DMA supply rate, while only x0 gates startup.
            if NCH == 3:
                segs = [(0, 0, 8), (1, 0, 8), (0, 8, HT), (1, 8, HT), (2, 0, HT)]
            else:
                segs = [(ci, 0, HT) for ci in range(NCH)]
            for ci, h0, h1 in segs:
                c0, cnn = chunks[ci]
                for ht in range(h0, h1):
                    ps = psp.tile(
                        [P, 512], f32, tag="ps", name=f"ps1_{ht}_{ci}"
                    )[:, :cnn]
                    n = 0
                    for wi, xi in ((0, 0), (1, 0), (0, 1)):
                        for j in range(DK // 2):
                            nc.tensor.matmul(
                                ps,
                                w1_sb[:, ht, wi, 2 * j : 2 * j + 2, :],
                                x_sb[:, ci, xi, 2 * j : 2 * j + 2, :],
                                start=(n == 0),
                                stop=(n == 3 * DK // 2 - 1),
                                perf_mode=DR,
                            )
                            n += 1
                    tmp = tmpp.tile([P, 512], f32, tag="tmp", name=f"t_{ht}_{ci}")[
                        :, :cnn
                    ]
                    nc.scalar.activation(
                        tmp,
                        ps,
                        mybir.ActivationFunctionType.Relu,
                        bias=b1_sb[:, ht : ht + 1],
                        scale=float(2.0 ** (HS - SW)),
                    )
                    nc.vector.tensor_copy(hh_sb[:, ht, c0 : c0 + cnn], tmp)
                    nc.vector.tensor_tensor(
                        hl_sb[:, ht, c0 : c0 + cnn],
                        tmp,
                        hh_sb[:, ht, c0 : c0 + cnn],
                        mybir.AluOpType.subtract,
                    )

            # Phase 2: y[ot] = 2^-(HS+SW) * ps2 + b2,  ps2 = 2^(HS+SW) h@W2
            # The very last piece is a small separate PSUM group so the final
            # epilogue + output DMA chain is short.
            for ot in range(OT):
                y_sb = yp.tile([P, C], f32, tag="y", name=f"y_{ot}")
                pieces = list(reversed(chunks))
                if ot == OT - 1:
                    c0l, cnl = pieces.pop()
                    cut = (cnl * 3 // 4) // 16 * 16
                    pieces += [(c0l, cut), (c0l + cut, cnl - cut)]
                for ci, (c0, cnn) in enumerate(pieces):
                    ps = psp.tile(
                        [P, 512], f32, tag="ps", name=f"ps2_{ot}_{ci}"
                    )[:, :cnn]
                    n = 0
                    for wi, hsb in ((0, hh_sb), (1, hh_sb), (0, hl_sb)):
                        for j in range(HK // 2):
                            nc.tensor.matmul(
                                ps,
                                w2_sb[:, ot, wi, 2 * j : 2 * j + 2, :],
                                hsb[:, 2 * j : 2 * j + 2, c0 : c0 + cnn],
                                start=(n == 0),
                                stop=(n == 3 * HK // 2 - 1),
                                perf_mode=DR,
                            )
                            n += 1
                    nc.scalar.activation(
                        y_sb[:, c0 : c0 + cnn],
                        ps,
                        mybir.ActivationFunctionType.Identity,
                        bias=b2_sb[:, ot : ot + 1],
                        scale=float(2.0 ** (-HS - SW)),
                    )
                    if ot == OT - 1:
                        # stagger the final ot's output per piece
                        nc.scalar.dma_start(
                            yT[ot * P : (ot + 1) * P, c0 : c0 + cnn],
                            y_sb[:, c0 : c0 + cnn],
                        )
                if ot < OT - 1:
                    nc.scalar.dma_start(yT[ot * P : (ot + 1) * P, :], y_sb[:])

    nc.compile()
    return nc


LAST_BUILD_KEY = None


def _get_built(C):
    global LAST_BUILD_KEY
    key = (C,)
    if key not in _BUILD_CACHE:
        _BUILD_CACHE[key] = _build(C)
    LAST_BUILD_KEY = key
    return _BUILD_CACHE[key]


_RUNNER_CACHE = {}
_WEIGHT_CACHE = {}


def _get_runner(C):
    """Reusable jitted SPMD executable for the bass program (compile once)."""
    key = (C,)
    if key in _RUNNER_CACHE:
        return _RUNNER_CACHE[key]

    import jax
    import concourse.mybir as mybir
    from concourse import bass2jax
    from jax.experimental.shard_map import shard_map
    from jax.sharding import Mesh, NamedSharding, PartitionSpec

    nc = _get_built(C)
    bass2jax.install_neuronx_cc_hook()

    partition_name = (
        nc.partition_id_tensor.name if nc.partition_id_tensor else None
    )
    in_names, out_names, out_avals = [], [], []
    for alloc in nc.m.functions[0].allocations:
        if not isinstance(alloc, mybir.MemoryLocationSet):
            continue
        name = alloc.memorylocations[0].name
        if alloc.kind == "ExternalInput":
            if name != partition_name:
                in_names.append(name)
        elif alloc.kind == "ExternalOutput":
            out_names.append(name)
            out_avals.append(
                jax.core.ShapedArray(
                    tuple(alloc.tensor_shape), mybir.dt.np(alloc.dtype)
                )
            )
    all_names = list(in_names) + list(out_names) + (
        [partition_name] if partition_name else []
    )

    def _body(*args):
        operands = list(args)
        if partition_name is not None:
            operands.append(bass2jax.partition_id_tensor())
        outs = bass2jax._bass_exec_p.bind(
            *operands,
            out_avals=tuple(out_avals),
            in_names=tuple(all_names),
            out_names=tuple(out_names),
            lowering_input_output_aliases=(),
            sim_require_finite=True,
            sim_require_nnan=True,
            nc=nc,
        )
        return tuple(outs)

    devices = jax.devices()[:E]
    mesh = Mesh(np.asarray(devices), ("core",))
    n_io = len(in_names) + len(out_names)
    fn = jax.jit(
        shard_map(
            _body,
            mesh=mesh,
            in_specs=(PartitionSpec("core"),) * n_io,
            out_specs=(PartitionSpec("core"),) * len(out_names),
            check_rep=False,
        ),
        keep_unused=True,
    )
    sharding = NamedSharding(mesh, PartitionSpec("core"))
    # Zero-filled output parameter buffers, device-resident. Not donated: the
    # kernel writes every element of its outputs, so reuse across calls is
    # safe.
    zeros = [
        jax.device_put(
            np.zeros((E * av.shape[0], *av.shape[1:]), av.dtype), sharding
        )
        for av in out_avals
    ]
    runner = {
        "fn": fn,
        "in_names": in_names,
        "out_names": out_names,
        "sharding": sharding,
        "zeros": zeros,
    }
    _RUNNER_CACHE[key] = runner
    return runner


def _weights_fingerprint(arrays):
    import hashlib

    h = hashlib.sha1()
    for k in sorted(arrays):
        a = np.ascontiguousarray(arrays[k])
        h.update(k.encode())
        h.update(str(a.shape).encode())
        flat = a.view(np.uint8).reshape(-1)
        h.update(flat[:: max(1, flat.size // 262144)].tobytes())  # ~256KB sample
        h.update(flat[-4096:].tobytes())
    return h.hexdigest()


def _device_weights(runner, key, arrays):
    """device_put the per-core-stacked weight arrays once, keyed by content."""
    import jax

    fp = (key, _weights_fingerprint(arrays))
    if fp not in _WEIGHT_CACHE:
        _WEIGHT_CACHE.clear()  # keep at most one weight set resident
        _WEIGHT_CACHE[fp] = {
            k: jax.device_put(v, runner["sharding"]) for k, v in arrays.items()
        }
    return _WEIGHT_CACHE[fp]


def _route(x, Wg, bg):
    """Host gating in float64; returns per-expert token ids and gate weights."""
    logits = x.astype(np.float64) @ Wg.astype(np.float64) + bg.astype(np.float64)
    order = np.argsort(-logits, axis=1, kind="stable")
    top2 = order[:, :TOPK]  # [T, 2]
    v = np.take_along_axis(logits, top2, axis=1)
    ex = np.exp(v - v.max(axis=1, keepdims=True))
    g = (ex / ex.sum(axis=1, keepdims=True)).astype(np.float32)  # [T, 2]
    ids, gates = [], []
    for e in range(E):
        sel = top2 == e  # [T, 2]
        te = np.where(sel.any(axis=1))[0]
        ge = np.where(sel[te, 0], g[te, 0], g[te, 1])
        ids.append(te)
        gates.append(ge.astype(np.float32))
    return ids, gates


def _f8():
    import ml_dtypes

    return np.dtype(ml_dtypes.float8_e4m3)


def _split_f8(a):
    """Return (hi, lo) e4m3 arrays with hi + lo ~= a."""
    f8 = _f8()
    hi = a.astype(f8)
    lo = (a - hi.astype(np.float32)).astype(f8)
    return hi, lo


def _prep_weights(W1, b1, W2, b2):
    """Quantize + lay out weights for the kernel, stacked per core.

    w1 tile layout: [p, ht, dk, m] = W1s[dk*128+p, ht*128+m]
    w2 tile layout: [p, ot, hk, m] = W2s[hk*128+p, ot*128+m]
    """
    s = np.float32(2.0**SW)
    arrs = {}
    # [E, D, H] -> [E, dk, p, ht, m] -> [E*p, ht, dk, m]
    W1s = (W1 * s).reshape(E, DK, P, HT, P)
    W2s = (W2 * s).reshape(E, HK, P, OT, P)
    for name, Ws in (("w1", W1s), ("w2", W2s)):
        hi, lo = _split_f8(Ws.astype(np.float32))
        # [e, k_tiles, p, out_tiles, m] -> [e, p, out_tiles, {hi,lo}, k_tiles, m]
        t = np.stack(
            (hi.transpose(0, 2, 3, 1, 4), lo.transpose(0, 2, 3, 1, 4)), axis=3
        )
        arrs[name] = np.ascontiguousarray(
            t.reshape(E * P, t.shape[2], 2, t.shape[4], P)
        )
    arrs["b1s"] = np.ascontiguousarray(
        (b1 * np.float32(2.0**HS)).reshape(E, HT, P).transpose(0, 2, 1).reshape(E * P, HT)
    ).astype(np.float32)
    arrs["b2s"] = np.ascontiguousarray(
        b2.reshape(E, OT, P).transpose(0, 2, 1).reshape(E * P, OT)
    ).astype(np.float32)
    return arrs


def _is_axon():
    try:
        from concourse._compat import axon_active

        return bool(axon_active())
    except Exception:  # noqa: BLE001
        return False


def _build_x_global(C, ids, x):
    """Chunk-major fp8 hi/lo-packed x dispatch array, stacked per core.

    Returns x8_g of shape [E*NCH, P, 2, DK, cn]; core e's slice is
    [e*NCH:(e+1)*NCH] with layout [chunk, p, hi/lo, dk, c].
    """
    cn = C // NCH
    f8 = _f8()
    x8_g = np.zeros((E, NCH, P, 2, DK, cn), f8)
    for e in range(E):
        te = ids[e]
        if len(te) == 0:
            continue
        xt = np.zeros((C, DK, P), np.float32)
        xt[: len(te)] = x[te].reshape(len(te), DK, P)
        hi, lo = _split_f8(xt)
        # [C, dk, p] -> [nch, cn, dk, p] -> [nch, p, dk, cn]
        x8_g[e, :, :, 0] = hi.reshape(NCH, cn, DK, P).transpose(0, 3, 2, 1)
        x8_g[e, :, :, 1] = lo.reshape(NCH, cn, DK, P).transpose(0, 3, 2, 1)
    return np.ascontiguousarray(x8_g.reshape(E * NCH, P, 2, DK, cn))


def _run_axon(C, ids, x, warrs):
    """Fast path: cached jitted SPMD executable, device-resident weights."""
    import jax

    runner = _get_runner(C)
    dev_w = _device_weights(runner, (C,), warrs)

    x8_g = _build_x_global(C, ids, x)
    x8_dev = jax.device_put(x8_g, runner["sharding"])

    operands = []
    for name in runner["in_names"]:
        if name == "x8":
            operands.append(x8_dev)
        else:
            operands.append(dev_w[name])
    operands.extend(runner["zeros"])
    outs = runner["fn"](*operands)
    return np.asarray(outs[runner["out_names"].index("yT")])  # [E*O, C]


def _run_native(C, ids, x, warrs):
    """Fallback for non-axon environments: bass_utils native NRT runner."""
    from concourse.bass_utils import run_bass_kernel_spmd

    nc = _get_built(C)
    x8_g = _build_x_global(C, ids, x)
    in_maps = []
    for e in range(E):
        m = {
            "x8": np.ascontiguousarray(x8_g[e * NCH : (e + 1) * NCH]),
        }
        for k, v in warrs.items():
            m[k] = np.ascontiguousarray(v[e * P : (e + 1) * P])
        in_maps.append(m)
    res = run_bass_kernel_spmd(nc, in_maps, core_ids=list(range(E)))
    return np.concatenate([res.results[e]["yT"] for e in range(E)], axis=0)


# Above this capacity the working set (x + h + y tiles at current pool
# depths) overflows SBUF; heavier routing skew runs as multiple batches.
_MAX_C = 1920

FALLBACK_USED = False  # set when the numpy emergency path ran (device down)


def _run_device(C, bids, x, warrs, W1, b1, W2, b2):
    """Run the bass kernel on the 8 cores, with one retry after a device
    error and a loud numpy fallback if the accelerator is unrecoverable."""
    for attempt in range(2):
        try:
            if _is_axon():
                return _run_axon(C, bids, x, warrs)
            return _run_native(C, bids, x, warrs)
        except Exception as ex:  # noqa: BLE001
            print(
                f"kernel: device run failed (attempt {attempt}): "
                f"{type(ex).__name__}: {str(ex)[:200]}",
                flush=True,
            )
            # Device arrays / executables may be poisoned; rebuild them.
            _RUNNER_CACHE.clear()
            _WEIGHT_CACHE.clear()
            try:
                import jax

                jax.clear_caches()
            except Exception:  # noqa: BLE001
                pass
    global FALLBACK_USED
    FALLBACK_USED = True
    print(
        "kernel: WARNING - accelerator unavailable after retries; "
        "computing this batch on the host (numpy) so the result is correct",
        flush=True,
    )
    yT_g = np.zeros((E * O, C), np.float32)
    for e in range(E):
        te = bids[e]
        if len(te) == 0:
            continue
        h = np.maximum(x[te] @ W1[e] + b1[e], 0.0)
        yT_g[e * O : (e + 1) * O, : len(te)] = (h @ W2[e] + b2[e]).T
    return yT_g


def kernel(x, Wg, bg, W1, b1, W2, b2):
    x = np.ascontiguousarray(np.asarray(x, np.float32))
    Wg = np.asarray(Wg, np.float32)
    bg = np.asarray(bg, np.float32)
    W1 = np.ascontiguousarray(np.asarray(W1, np.float32))
    b1 = np.ascontiguousarray(np.asarray(b1, np.float32))
    W2 = np.ascontiguousarray(np.asarray(W2, np.float32))
    b2 = np.ascontiguousarray(np.asarray(b2, np.float32))

    assert x.shape[1] == D and Wg.shape == (D, E)
    assert W1.shape == (E, D, H) and W2.shape == (E, H, O)

    ids, gates = _route(x, Wg, bg)

    warrs = _prep_weights(W1, b1, W2, b2)

    out = np.zeros((x.shape[0], O), np.float32)
    max_load = max(len(te) for te in ids)
    n_batches = -(-max_load // _MAX_C)
    for b in range(n_batches):
        bids = [te[b * _MAX_C : (b + 1) * _MAX_C] for te in ids]
        C = _capacity(max(len(te) for te in bids))
        yT_g = _run_device(C, bids, x, warrs, W1, b1, W2, b2)
        for e in range(E):
            te = bids[e]
            ge = gates[e][b * _MAX_C : (b + 1) * _MAX_C]
            ye = yT_g[e * O : e * O + O, : len(te)].T  # [n_e, O]
            out[te] += ge[:, None] * ye
    return out
